# revision 24
# baseline (speedup 1.0000x reference)
"""Trainium2 Bass kernel for a custom Jacobi-basis layer.

Math:
    t = tanh(x)                                  x: [B, I] f32
    J[b,i,k] = P_k^(1,1)(t[b,i])                 Jacobi polys, k = 0..8
    out[b,o] = sum_{i,k} J[b,i,k] * coeff[o,i,k] * weights[o,i]

Strategy (8 NeuronCores, data-parallel over batch):
  * Fold weights into coeff on host: Cw[o,i,k] = coeff[o,i,k]*weights[o,i].
  * Replace the Jacobi basis with a Chebyshev-like basis phi_m(t) that is
    generable almost entirely on the Scalar (ACT) engine:
        phi1 = t            (tanh)
        phi2 = (sqrt2*t)^2          = 2t^2            [ACT square]
        phi3 = (phi2-1.5)*t                           [DVE stt]
        phi4 = (phi2-1.5)^2                           [ACT square]
        phi5 = (phi2-1.0)*phi3                        [DVE stt]
        phi6 = (2*phi3)^2                             [ACT square]
        phi7 = (phi4-0.75)*phi3                       [DVE stt]
        phi8 = (phi4-1.1)^2                           [ACT square]
    The exact change of basis J_k = sum_m C[k,m] phi_m is folded into the
    matmul operand on host (f64 poly algebra).  The phi's track scaled
    Chebyshev polynomials, so the folded operand R'_m stays O(1) and the
    fp16 quantization error is ~3e-3 (vs 1.5e-2 for a raw monomial basis).
    This removes the serial fp32 recurrence + per-plane fp16 casts that
    made DVE/ACT each ~30us busy; now ACT ~6us, DVE ~4us, and the kernel
    is purely PE-bound (128 fp16 [128x128]@[128x512] matmuls ~= 27.6us).
  * The m=0 (constant) term is a per-output bias that is independent of x;
    it is added on the host after the gather (saves 4 PE matmuls and the
    consts DMA).
  * Pre-tile raw warmup: a dummy DMA on each hardware DMA ring absorbs the
    ~0.9us ring spin-up, a dummy activation hoists the 1.3us ACT-table
    load before the tile entry gate, and junk matmuls (raw + in-tile)
    release the PE HAM clock gate before the real stream begins.
  * x is shipped as fp16, the first r plane is split into 4 ic-chunks, and
    DMA issue is spread across the Sync / GpSimd queues.
  * Planes 7 and 8 run b-tile-major so each b-tile's psum finishes early;
    psum->out copies are split across Scalar/Vector and the fp16 output
    chunks stream on both DMA rings while the last matmuls still run.
    Host upcasts the fp16 output to f32.
"""

import numpy as np

import concourse.mybir as mybir
import concourse.tile as tile
from concourse import bacc
from concourse.bass_utils import run_bass_kernel_spmd

ORDER = 8
B, I, O = 4096, 512, 512
NCORES = 8
BC = B // NCORES          # batch rows per core = 512
P = 128                   # partitions
NIC = I // P              # i-chunks = 4
BT = BC // P              # b-tiles per core = 4
FREE = NIC * BC           # free dim of basis planes = 2048
SQRT2 = 1.4142135623730951

# basis shaping constants (see docstring); values chosen so phi_m ~ O(1)
C3 = -1.5   # phi3 = (phi2 + C3) * t,   phi4 = (phi2 + C3)^2
C5 = -1.0   # phi5 = (phi2 + C5) * phi3
C7 = -0.75  # phi7 = (phi4 + C7) * phi3
C8 = -1.1   # phi8 = (phi4 + C8)^2


def _basis_change():
    """Exact matrix C with J_k(t) = sum_m C[k,m] phi_m(t), f64 poly algebra."""
    Pp = np.polynomial.polynomial
    a = b = 1.0
    p1 = np.array([0.0, 1.0])
    p2 = Pp.polymul([0.0, SQRT2], [0.0, SQRT2])
    p2s = Pp.polyadd(p2, [C3])
    p3 = Pp.polymul(p2s, p1)
    p4 = Pp.polymul(p2s, p2s)
    p5 = Pp.polymul(Pp.polyadd(p2, [C5]), p3)
    p6 = Pp.polymul(Pp.polymul([2.0], p3), Pp.polymul([2.0], p3))
    p7 = Pp.polymul(Pp.polyadd(p4, [C7]), p3)
    p8 = Pp.polymul(Pp.polyadd(p4, [C8]), Pp.polyadd(p4, [C8]))
    basis = [np.array([1.0]), p1, p2, p3, p4, p5, p6, p7, p8]
    Mb = np.zeros((9, 9))
    for m, p in enumerate(basis):
        Mb[m, : len(p)] = p
    polys = [np.array([1.0]), np.array([0.0, 2.0])]
    for i in range(2, ORDER + 1):
        k1 = (2 * i + a + b) * (2 * i + a + b - 1) / (2 * i * (i + a + b))
        k3 = (i + a - 1) * (i + b - 1) * (2 * i + a + b) / (
            i * (i + a + b) * (2 * i + a + b - 2)
        )
        polys.append(
            Pp.polysub(Pp.polymul([0.0, k1], polys[-1]), Pp.polymul([k3], polys[-2]))
        )
    MJ = np.zeros((9, 9))
    for k, p in enumerate(polys):
        MJ[k, : len(p)] = p
    return MJ @ np.linalg.inv(Mb)


def _build_module():
    nc = bacc.Bacc("TRN2", num_devices=NCORES)
    f32 = mybir.dt.float32
    f16 = mybir.dt.float16

    # xt chunk-major: [ic, p, BC]; xt[ic, p, b] = x[core*BC+b, ic*128+p], fp16
    xt_d = nc.dram_tensor("xt", [NIC, P, BC], f16, kind="ExternalInput")
    # r layout: [p, (m-1)*FREE + ic*O + o] = R'_m[o, ic*128+p], fp16
    r_d = nc.dram_tensor("r", [P, ORDER * FREE], f16, kind="ExternalInput")
    # out layout: [p, bt*O + o] = unbiased output[core*BC + bt*128 + p, o], fp16
    out_d = nc.dram_tensor("out", [P, BT * O], f16, kind="ExternalOutput")

    mult = mybir.AluOpType.mult
    add = mybir.AluOpType.add
    Square = mybir.ActivationFunctionType.Square
    Tanh = mybir.ActivationFunctionType.Tanh

    from concourse.tile_rust import add_dep_helper

    H = FREE // 2
    halves = (slice(0, H), slice(H, FREE))

    # ---- raw pre-tile warmup: hoist the ACT-table load (1.3us) so it
    # overlaps the tile entry gate instead of gating the first tanh.
    # (Scratch contents are irrelevant; the read is never consumed.)
    act_scr = nc.alloc_sbuf_tensor("act_scr", [P, 1], f32)
    nc.scalar.activation(act_scr.ap(), act_scr.ap(), Tanh)
    junk_sb = nc.alloc_sbuf_tensor("junk_sb", [P, P + O], f16)
    ps_junk = nc.alloc_psum_tensor("ps_junk", [P, O], f32)

    with tile.TileContext(nc) as tc:
        with (
            tc.tile_pool(name="io", bufs=1) as io,
            tc.tile_pool(name="psum", bufs=1, space="PSUM") as pp,
        ):
            def ics(ap, ic):
                return ap[:, ic * BC : (ic + 1) * BC]

            # --- input DMAs.  The DMA engines round-robin packets across ALL
            # queued descriptors, so a chunk lands early only if little else
            # is queued; dep-laddered DMA->DMA hops cost ~3us each (sem +
            # issue latency).  Scheme: keep only x + r1 + r2 (~1.5MB) queued
            # up front, and pace r3..r8 with COMPUTE-progress gates (added
            # retroactively after the matmul stream is emitted) so the DMA
            # stream stays ~1.5 planes ahead of the PE without contention.
            x_t = io.tile([P, FREE], f16, tag="x")
            r_t = [io.tile([P, FREE], f16, tag=f"r{m}", name=f"r{m}")
                   for m in range(1, ORDER + 1)]

            def rsl(m, lo, hi):
                return r_d[:, (m - 1) * FREE + lo : (m - 1) * FREE + hi]

            HB = BC // 2  # 256
            # heads on the sync queue (its ring starts ~0.6us before the
            # gpsimd ring): first x quarter-chunk (64KB, gates tanh) and the
            # first r1 ic-chunk (gates the first real matmul)
            nc.sync.dma_start(x_t[:, 0:HB], xt_d[0][:, 0:HB])
            d_r1 = [nc.sync.dma_start(r_t[0][:, 0:O], rsl(1, 0, O))]
            nc.sync.dma_start(x_t[:, HB:BC], xt_d[0][:, HB:BC])
            for ic in range(1, NIC):
                nc.sync.dma_start(ics(x_t, ic), xt_d[ic])
            for ic in range(1, NIC):
                d_r1.append(nc.gpsimd.dma_start(
                    r_t[0][:, ic * O : (ic + 1) * O],
                    rsl(1, ic * O, (ic + 1) * O)))
            # small consts for ACT square biases, needed by ~11us
            c3_t = io.tile([P, 1], f32, tag="c3")
            c8_t = io.tile([P, 1], f32, tag="c8")
            nc.gpsimd.memset(c3_t[:], C3)
            nc.gpsimd.memset(c8_t[:], C8)
            # r2 free-flows behind the x+r1 early set (~1.5MB total); r3..r8
            # are each held back by a SINGLE-hop dep on an early DMA (a hop
            # costs ~2.8us of sem+issue latency, so no chains off late DMAs)
            d_late = {2: nc.gpsimd.dma_start(r_t[1][:], rsl(2, 0, FREE))}
            gates = {3: d_r1[1], 4: d_r1[3], 5: d_late[2]}
            for m in range(3, ORDER + 1):
                d = nc.gpsimd.dma_start(r_t[m - 1][:], rsl(m, 0, FREE))
                add_dep_helper(d.ins, gates[m].ins, reason="dma pacing")
                d_late[m] = d
                if m + 3 <= ORDER:
                    gates[m + 3] = d

            # --- junk matmuls release the PE HAM clock gate while the
            # tanh / r1 pipeline fills (junk data is never read); they must
            # bridge gap-free into the real stream or the HAM ramp resets ---
            for _ in range(5):
                nc.tensor.matmul(
                    ps_junk.ap(), junk_sb.ap()[:, 0:P], junk_sb.ap()[:, P : P + O],
                    start=True, stop=True,
                )

            # --- basis planes (all fp16) ---
            ph = [None] * (ORDER + 1)
            for m in range(1, ORDER + 1):
                ph[m] = io.tile([P, FREE], f16, tag=f"ph{m}", name=f"ph{m}")
            t16 = ph[1]
            nc.scalar.activation(t16[:, 0:HB], x_t[:, 0:HB], Tanh)
            nc.scalar.activation(t16[:, HB:BC], x_t[:, HB:BC], Tanh)
            for ic in range(1, NIC):
                nc.scalar.activation(ics(t16, ic), ics(x_t, ic), Tanh)
            for h in (0, 1):
                sl = halves[h]
                nc.scalar.activation(ph[2][:, sl], t16[:, sl], Square, scale=SQRT2)
            for h in (0, 1):
                sl = halves[h]
                nc.vector.scalar_tensor_tensor(
                    ph[3][:, sl], ph[2][:, sl], C3, t16[:, sl], add, mult)
            for h in (0, 1):
                sl = halves[h]
                nc.scalar.activation(ph[4][:, sl], ph[2][:, sl], Square,
                                     bias=c3_t[:])
            for h in (0, 1):
                sl = halves[h]
                nc.vector.scalar_tensor_tensor(
                    ph[5][:, sl], ph[2][:, sl], C5, ph[3][:, sl], add, mult)
            for h in (0, 1):
                sl = halves[h]
                nc.scalar.activation(ph[6][:, sl], ph[3][:, sl], Square, scale=2.0)
            for h in (0, 1):
                sl = halves[h]
                nc.vector.scalar_tensor_tensor(
                    ph[7][:, sl], ph[4][:, sl], C7, ph[3][:, sl], add, mult)
            for h in (0, 1):
                sl = halves[h]
                nc.scalar.activation(ph[8][:, sl], ph[4][:, sl], Square,
                                     bias=c8_t[:])

            # --- matmul stream: psum[bt] += sum_{m,ic} phi_m_blk^T @ R'_m_blk
            psums = [
                pp.tile([P, O], f32, tag=f"ps{bt}", name=f"ps{bt}")
                for bt in range(BT)
            ]
            out_t = io.tile([P, BT * O], f16, tag="out")
            for m in range(1, ORDER - 1):
                for ic in range(NIC):
                    for bt in range(BT):
                        col = ic * BC + bt * P
                        nc.tensor.matmul(
                            psums[bt][:], ph[m][:, col : col + P],
                            r_t[m - 1][:, ic * O : (ic + 1) * O],
                            start=(m == 1 and ic == 0), stop=False,
                        )
            # last two planes b-tile-major: each b-tile's psum finishes ~1.7us
            # apart, so copies + out DMA stream under the remaining matmuls
            HO = O // 2
            for bt in range(BT):
                for m in (ORDER - 1, ORDER):
                    for ic in range(NIC):
                        col = ic * BC + bt * P
                        nc.tensor.matmul(
                            psums[bt][:], ph[m][:, col : col + P],
                            r_t[m - 1][:, ic * O : (ic + 1) * O],
                            start=False, stop=(m == ORDER and ic == NIC - 1),
                        )
                lo = bt * O
                # split the psum->fp16 copy across Scalar and Vector, and
                # alternate out chunks across both DMA queues so no out
                # issue ever queues behind another
                nc.scalar.copy(out_t[:, lo : lo + HO], psums[bt][:, 0:HO])
                nc.vector.tensor_copy(out_t[:, lo + HO : lo + O], psums[bt][:, HO:O])
                eng = nc.sync if bt % 2 == 0 else nc.gpsimd
                eng.dma_start(out_d[:, lo : lo + O], out_t[:, lo : lo + O])
    nc.compile()
    return nc


def _prep_operands(weights, coeff):
    """Host-side, input-independent preprocessing of the layer constants."""
    C = _basis_change()
    Cw = coeff.astype(np.float64) * weights.astype(np.float64)[:, :, None]
    Rm = np.einsum("oik,km->oim", Cw, C)            # [O, I, 9] in phi basis
    bias = Rm[:, :, 0].sum(axis=1).astype(np.float32)   # [O], added on host
    r = np.empty((ORDER, P, FREE), dtype=np.float32)
    for m in range(1, ORDER + 1):
        tmp = Rm[:, :, m].T.astype(np.float32)       # [I, O]
        r[m - 1] = tmp.reshape(NIC, P, O).transpose(1, 0, 2).reshape(P, FREE)
    r = np.ascontiguousarray(
        r.transpose(1, 0, 2).reshape(P, ORDER * FREE)
    ).astype(np.float16)
    return r, bias


def _prep_x(x):
    """Per-core [NIC, 128, BC] fp16 views: xt[ic, p, b] = x[core*BC+b, ic*128+p]."""
    shards = []
    for core in range(NCORES):
        xc = np.ascontiguousarray(
            x[core * BC : (core + 1) * BC, :].T.astype(np.float16)
        )  # [I, BC]
        shards.append(np.ascontiguousarray(xc.reshape(NIC, P, BC)))
    return shards


def _install_ntff_hook():
    """Register the NTFF profile hook that the image's boot skips (no
    antenv.axon_hooks module). Same ctypes ABI as trn_boot's
    _ntff_profile_via_ctypes. Only used for traced (profiling) runs."""
    import sys
    import types
    import ctypes
    import contextlib

    if "antenv.axon_hooks" in sys.modules:
        return
    mod = types.ModuleType("antenv.axon_hooks")
    state = {"hook": None}
    mod.set_axon_ntff_profile_hook = lambda h: state.__setitem__("hook", h)
    mod.get_axon_ntff_profile_hook = lambda: state["hook"]
    sys.modules["antenv.axon_hooks"] = mod
    import antenv

    antenv.axon_hooks = mod

    so_path = "/opt/axon/libaxon_pjrt.so"
    lib = ctypes.CDLL(so_path)
    if not hasattr(lib, "axon_start_nrt_profile"):
        return
    lib.axon_start_nrt_profile.argtypes = [
        ctypes.POINTER(ctypes.c_int64),
        ctypes.c_size_t,
    ]
    lib.axon_start_nrt_profile.restype = ctypes.c_int64
    lib.axon_stop_nrt_profile.argtypes = [ctypes.c_char_p]
    lib.axon_stop_nrt_profile.restype = ctypes.c_int64

    @contextlib.contextmanager
    def _hook(output_dir, device_ids):
        import jax

        jax.devices()
        if device_ids:
            ids = (ctypes.c_int64 * len(device_ids))(*device_ids)
            rc = lib.axon_start_nrt_profile(ids, len(device_ids))
        else:
            rc = lib.axon_start_nrt_profile(None, 0)
        if rc != 0:
            raise RuntimeError(f"axon_start_nrt_profile rc={rc}")
        try:
            yield
        finally:
            n = lib.axon_stop_nrt_profile(str(output_dir).encode())
            print(f"ntff profile: {n} file(s) written to {output_dir}")

    mod.set_axon_ntff_profile_hook(_hook)


_NC_CACHE = None


def _get_module():
    global _NC_CACHE
    if _NC_CACHE is None:
        _NC_CACHE = _build_module()
    return _NC_CACHE


def _run(x, weights, coeff, trace=False):
    nc = _get_module()
    r, bias = _prep_operands(weights, coeff)
    xs = _prep_x(np.asarray(x, dtype=np.float32))
    in_maps = [{"xt": xs[core], "r": r} for core in range(NCORES)]
    try:
        res = run_bass_kernel_spmd(
            nc, in_maps, core_ids=list(range(NCORES)), trace=trace
        )
    except Exception:
        res = run_bass_kernel_spmd(
            nc, in_maps, core_ids=list(range(NCORES)), trace=trace
        )
    out = np.concatenate(
        [
            res.results[core]["out"]
            .astype(np.float32)
            .reshape(P, BT, O)
            .transpose(1, 0, 2)
            .reshape(BC, O)
            for core in range(NCORES)
        ],
        axis=0,
    )
    out += bias[None, :]
    return out, res


def kernel(x, weights, coeff):
    out, _ = _run(x, weights, coeff, trace=False)
    return out


def kernel_traced(x, weights, coeff):
    _install_ntff_hook()
    out, res = _run(x, weights, coeff, trace=True)
    return out, res


# revision 33
# speedup vs baseline: 1.0622x; 1.0622x over previous
"""Trainium2 Bass kernel for a custom Jacobi-basis layer.

Math:
    t = tanh(x)                                  x: [B, I] f32
    J[b,i,k] = P_k^(1,1)(t[b,i])                 Jacobi polys, k = 0..8
    out[b,o] = sum_{i,k} J[b,i,k] * coeff[o,i,k] * weights[o,i]

Strategy (8 NeuronCores, data-parallel over batch):
  * Fold weights into coeff on host: Cw[o,i,k] = coeff[o,i,k]*weights[o,i].
  * Replace the Jacobi basis with a Chebyshev-like basis phi_m(t) that is
    generable almost entirely on the Scalar (ACT) engine:
        phi1 = t            (tanh)
        phi2 = (sqrt2*t)^2          = 2t^2            [ACT square]
        phi3 = (phi2-1.5)*t                           [DVE stt]
        phi4 = (phi2-1.5)^2                           [ACT square]
        phi5 = (phi2-1.0)*phi3                        [DVE stt]
        phi6 = (2*phi3)^2                             [ACT square]
        phi7 = (phi4-0.75)*phi3                       [DVE stt]
        phi8 = (phi4-1.1)^2                           [ACT square]
    The exact change of basis J_k = sum_m C[k,m] phi_m is folded into the
    matmul operand on host (f64 poly algebra).  The phi's track scaled
    Chebyshev polynomials, so the folded operand R'_m stays O(1) and the
    fp16 quantization error is ~3e-3 (vs 1.5e-2 for a raw monomial basis).
    This removes the serial fp32 recurrence + per-plane fp16 casts that
    made DVE/ACT each ~30us busy; now ACT ~6us, DVE ~4us, and the kernel
    is purely PE-bound (128 fp16 [128x128]@[128x512] matmuls ~= 27.6us).
  * The m=0 (constant) term is a per-output bias that is independent of x;
    it is added on the host after the gather (saves 4 PE matmuls and the
    consts DMA).
  * Pre-tile raw warmup: a dummy DMA on each hardware DMA ring absorbs the
    ~0.9us ring spin-up, a dummy activation hoists the 1.3us ACT-table
    load before the tile entry gate, and junk matmuls (raw + in-tile)
    release the PE HAM clock gate before the real stream begins.
  * x is shipped as fp16, the first r plane is split into 4 ic-chunks, and
    DMA issue is spread across the Sync / GpSimd queues.
  * Planes 7 and 8 run b-tile-major so each b-tile's psum finishes early;
    psum->out copies are split across Scalar/Vector and the fp16 output
    chunks stream on both DMA rings while the last matmuls still run.
    Host upcasts the fp16 output to f32.
"""

import numpy as np

import concourse.mybir as mybir
import concourse.tile as tile
from concourse import bacc
from concourse.bass_utils import run_bass_kernel_spmd

ORDER = 8
B, I, O = 4096, 512, 512
NCORES = 8
BC = B // NCORES          # batch rows per core = 512
P = 128                   # partitions
NIC = I // P              # i-chunks = 4
BT = BC // P              # b-tiles per core = 4
FREE = NIC * BC           # free dim of basis planes = 2048
SQRT2 = 1.4142135623730951

# basis shaping constants (see docstring); values chosen so phi_m ~ O(1)
C3 = -1.5   # phi3 = (phi2 + C3) * t,   phi4 = (phi2 + C3)^2
C5 = -1.0   # phi5 = (phi2 + C5) * phi3
C7 = -0.75  # phi7 = (phi4 + C7) * phi3
C8 = -1.1   # phi8 = (phi4 + C8)^2


def _basis_change():
    """Exact matrix C with J_k(t) = sum_m C[k,m] phi_m(t), f64 poly algebra."""
    Pp = np.polynomial.polynomial
    a = b = 1.0
    p1 = np.array([0.0, 1.0])
    p2 = Pp.polymul([0.0, SQRT2], [0.0, SQRT2])
    p2s = Pp.polyadd(p2, [C3])
    p3 = Pp.polymul(p2s, p1)
    p4 = Pp.polymul(p2s, p2s)
    p5 = Pp.polymul(Pp.polyadd(p2, [C5]), p3)
    p6 = Pp.polymul(Pp.polymul([2.0], p3), Pp.polymul([2.0], p3))
    p7 = Pp.polymul(Pp.polyadd(p4, [C7]), p3)
    p8 = Pp.polymul(Pp.polyadd(p4, [C8]), Pp.polyadd(p4, [C8]))
    basis = [np.array([1.0]), p1, p2, p3, p4, p5, p6, p7, p8]
    Mb = np.zeros((9, 9))
    for m, p in enumerate(basis):
        Mb[m, : len(p)] = p
    polys = [np.array([1.0]), np.array([0.0, 2.0])]
    for i in range(2, ORDER + 1):
        k1 = (2 * i + a + b) * (2 * i + a + b - 1) / (2 * i * (i + a + b))
        k3 = (i + a - 1) * (i + b - 1) * (2 * i + a + b) / (
            i * (i + a + b) * (2 * i + a + b - 2)
        )
        polys.append(
            Pp.polysub(Pp.polymul([0.0, k1], polys[-1]), Pp.polymul([k3], polys[-2]))
        )
    MJ = np.zeros((9, 9))
    for k, p in enumerate(polys):
        MJ[k, : len(p)] = p
    return MJ @ np.linalg.inv(Mb)


def _build_module():
    nc = bacc.Bacc("TRN2", num_devices=NCORES)
    f32 = mybir.dt.float32
    f16 = mybir.dt.float16

    # xt chunk-major: [ic, p, BC]; xt[ic, p, b] = x[core*BC+b, ic*128+p], fp16
    xt_d = nc.dram_tensor("xt", [NIC, P, BC], f16, kind="ExternalInput")
    # r layout: [p, (m-1)*FREE + ic*O + o] = R'_m[o, ic*128+p], fp16
    r_d = nc.dram_tensor("r", [P, ORDER * FREE], f16, kind="ExternalInput")
    # out layout: [p, bt*O + o] = unbiased output[core*BC + bt*128 + p, o], fp16
    out_d = nc.dram_tensor("out", [P, BT * O], f16, kind="ExternalOutput")

    mult = mybir.AluOpType.mult
    add = mybir.AluOpType.add
    Square = mybir.ActivationFunctionType.Square
    Tanh = mybir.ActivationFunctionType.Tanh

    from concourse.tile_rust import add_dep_helper

    H = FREE // 2
    halves = (slice(0, H), slice(H, FREE))

    # ---- raw pre-tile warmup: hoist the ACT-table load (1.3us) so it
    # overlaps the tile entry gate instead of gating the first tanh.
    # (Scratch contents are irrelevant; the read is never consumed.)
    act_scr = nc.alloc_sbuf_tensor("act_scr", [P, 1], f32)
    nc.scalar.activation(act_scr.ap(), act_scr.ap(), Tanh)
    junk_sb = nc.alloc_sbuf_tensor("junk_sb", [P, P + O], f16)
    ps_junk = nc.alloc_psum_tensor("ps_junk", [P, O], f32)


    with tile.TileContext(nc) as tc:
        with (
            tc.tile_pool(name="io", bufs=1) as io,
            tc.tile_pool(name="psum", bufs=1, space="PSUM") as pp,
        ):
            def ics(ap, ic):
                return ap[:, ic * BC : (ic + 1) * BC]

            # --- input DMAs.  The DMA engines round-robin packets across ALL
            # queued descriptors, so a chunk lands early only if little else
            # is queued; dep-laddered DMA->DMA hops cost ~3us each (sem +
            # issue latency).  Scheme: keep only x + r1 + r2 (~1.5MB) queued
            # up front, and pace r3..r8 with COMPUTE-progress gates (added
            # retroactively after the matmul stream is emitted) so the DMA
            # stream stays ~1.5 planes ahead of the PE without contention.
            x_t = io.tile([P, FREE], f16, tag="x")
            r_t = [io.tile([P, FREE], f16, tag=f"r{m}", name=f"r{m}")
                   for m in range(1, ORDER + 1)]

            def rsl(m, lo, hi):
                return r_d[:, (m - 1) * FREE + lo : (m - 1) * FREE + hi]

            HB = BC // 2  # 256
            # x stream on the sync queue (first quarter split so tanh starts
            # early), r1 ic-chunks on the gpsimd queue
            nc.sync.dma_start(x_t[:, 0:HB], xt_d[0][:, 0:HB])
            nc.sync.dma_start(x_t[:, HB:BC], xt_d[0][:, HB:BC])
            for ic in range(1, NIC):
                nc.sync.dma_start(ics(x_t, ic), xt_d[ic])
            d_r1 = []
            for ic in range(NIC):
                d_r1.append(nc.gpsimd.dma_start(
                    r_t[0][:, ic * O : (ic + 1) * O],
                    rsl(1, ic * O, (ic + 1) * O)))
            # small consts for ACT square biases, needed by ~11us
            c3_t = io.tile([P, 1], f32, tag="c3")
            c8_t = io.tile([P, 1], f32, tag="c8")
            nc.gpsimd.memset(c3_t[:], C3)
            nc.gpsimd.memset(c8_t[:], C8)
            # r2 free-flows behind the x+r1 early set (~1.5MB total); r3..r8
            # are each held back by a SINGLE-hop dep on an early DMA (a hop
            # costs ~2.8us of sem+issue latency, so no chains off late DMAs)
            d_late = {2: nc.gpsimd.dma_start(r_t[1][:], rsl(2, 0, FREE))}
            gates = {3: d_r1[1], 4: d_r1[3], 5: d_late[2]}
            for m in range(3, ORDER + 1):
                d = nc.gpsimd.dma_start(r_t[m - 1][:], rsl(m, 0, FREE))
                add_dep_helper(d.ins, gates[m].ins, reason="dma pacing")
                d_late[m] = d
                if m + 3 <= ORDER:
                    gates[m + 3] = d

            # --- junk matmuls release the PE HAM clock gate while the
            # tanh / r1 pipeline fills (junk data is never read); they must
            # bridge gap-free into the real stream or the HAM ramp resets ---
            for _ in range(9):
                nc.tensor.matmul(
                    ps_junk.ap(), junk_sb.ap()[:, 0:P], junk_sb.ap()[:, P : P + O],
                    start=True, stop=True,
                )

            # --- basis planes (all fp16) ---
            ph = [None] * (ORDER + 1)
            for m in range(1, ORDER + 1):
                ph[m] = io.tile([P, FREE], f16, tag=f"ph{m}", name=f"ph{m}")
            t16 = ph[1]
            nc.scalar.activation(t16[:, 0:HB], x_t[:, 0:HB], Tanh)
            nc.scalar.activation(t16[:, HB:BC], x_t[:, HB:BC], Tanh)
            for ic in range(1, NIC):
                nc.scalar.activation(ics(t16, ic), ics(x_t, ic), Tanh)
            for h in (0, 1):
                sl = halves[h]
                nc.scalar.activation(ph[2][:, sl], t16[:, sl], Square, scale=SQRT2)
            for h in (0, 1):
                sl = halves[h]
                nc.vector.scalar_tensor_tensor(
                    ph[3][:, sl], ph[2][:, sl], C3, t16[:, sl], add, mult)
            for h in (0, 1):
                sl = halves[h]
                nc.scalar.activation(ph[4][:, sl], ph[2][:, sl], Square,
                                     bias=c3_t[:])
            for h in (0, 1):
                sl = halves[h]
                nc.vector.scalar_tensor_tensor(
                    ph[5][:, sl], ph[2][:, sl], C5, ph[3][:, sl], add, mult)
            for h in (0, 1):
                sl = halves[h]
                nc.scalar.activation(ph[6][:, sl], ph[3][:, sl], Square, scale=2.0)
            for h in (0, 1):
                sl = halves[h]
                nc.vector.scalar_tensor_tensor(
                    ph[7][:, sl], ph[4][:, sl], C7, ph[3][:, sl], add, mult)
            for h in (0, 1):
                sl = halves[h]
                nc.scalar.activation(ph[8][:, sl], ph[4][:, sl], Square,
                                     bias=c8_t[:])

            # --- matmul stream: psum[bt] += sum_{m,ic} phi_m_blk^T @ R'_m_blk
            psums = [
                pp.tile([P, O], f32, tag=f"ps{bt}", name=f"ps{bt}")
                for bt in range(BT)
            ]
            out_t = io.tile([P, BT * O], f16, tag="out")
            for m in range(1, ORDER - 1):
                for ic in range(NIC):
                    for bt in range(BT):
                        col = ic * BC + bt * P
                        nc.tensor.matmul(
                            psums[bt][:], ph[m][:, col : col + P],
                            r_t[m - 1][:, ic * O : (ic + 1) * O],
                            start=(m == 1 and ic == 0), stop=False,
                        )
            # last two planes b-tile-major: each b-tile's psum finishes ~1.7us
            # apart, so copies + out DMA stream under the remaining matmuls
            HO = O // 2
            for bt in range(BT):
                for m in (ORDER - 1, ORDER):
                    for ic in range(NIC):
                        col = ic * BC + bt * P
                        nc.tensor.matmul(
                            psums[bt][:], ph[m][:, col : col + P],
                            r_t[m - 1][:, ic * O : (ic + 1) * O],
                            start=False, stop=(m == ORDER and ic == NIC - 1),
                        )
                lo = bt * O
                # split the psum->fp16 copy across Scalar and Vector, and
                # alternate out chunks across both DMA queues so no out
                # issue ever queues behind another
                nc.scalar.copy(out_t[:, lo : lo + HO], psums[bt][:, 0:HO])
                nc.vector.tensor_copy(out_t[:, lo + HO : lo + O], psums[bt][:, HO:O])
                eng = nc.sync if bt % 2 == 0 else nc.gpsimd
                eng.dma_start(out_d[:, lo : lo + O], out_t[:, lo : lo + O])
    nc.compile()
    return nc


def _prep_operands(weights, coeff):
    """Host-side, input-independent preprocessing of the layer constants."""
    C = _basis_change()
    Cw = coeff.astype(np.float64) * weights.astype(np.float64)[:, :, None]
    Rm = np.einsum("oik,km->oim", Cw, C)            # [O, I, 9] in phi basis
    bias = Rm[:, :, 0].sum(axis=1).astype(np.float32)   # [O], added on host
    r = np.empty((ORDER, P, FREE), dtype=np.float32)
    for m in range(1, ORDER + 1):
        tmp = Rm[:, :, m].T.astype(np.float32)       # [I, O]
        r[m - 1] = tmp.reshape(NIC, P, O).transpose(1, 0, 2).reshape(P, FREE)
    r = np.ascontiguousarray(
        r.transpose(1, 0, 2).reshape(P, ORDER * FREE)
    ).astype(np.float16)
    return r, bias


def _prep_x(x):
    """Per-core [NIC, 128, BC] fp16 views: xt[ic, p, b] = x[core*BC+b, ic*128+p]."""
    shards = []
    for core in range(NCORES):
        xc = np.ascontiguousarray(
            x[core * BC : (core + 1) * BC, :].T.astype(np.float16)
        )  # [I, BC]
        shards.append(np.ascontiguousarray(xc.reshape(NIC, P, BC)))
    return shards


def _install_ntff_hook():
    """Register the NTFF profile hook that the image's boot skips (no
    antenv.axon_hooks module). Same ctypes ABI as trn_boot's
    _ntff_profile_via_ctypes. Only used for traced (profiling) runs."""
    import sys
    import types
    import ctypes
    import contextlib

    if "antenv.axon_hooks" in sys.modules:
        return
    mod = types.ModuleType("antenv.axon_hooks")
    state = {"hook": None}
    mod.set_axon_ntff_profile_hook = lambda h: state.__setitem__("hook", h)
    mod.get_axon_ntff_profile_hook = lambda: state["hook"]
    sys.modules["antenv.axon_hooks"] = mod
    import antenv

    antenv.axon_hooks = mod

    so_path = "/opt/axon/libaxon_pjrt.so"
    lib = ctypes.CDLL(so_path)
    if not hasattr(lib, "axon_start_nrt_profile"):
        return
    lib.axon_start_nrt_profile.argtypes = [
        ctypes.POINTER(ctypes.c_int64),
        ctypes.c_size_t,
    ]
    lib.axon_start_nrt_profile.restype = ctypes.c_int64
    lib.axon_stop_nrt_profile.argtypes = [ctypes.c_char_p]
    lib.axon_stop_nrt_profile.restype = ctypes.c_int64

    @contextlib.contextmanager
    def _hook(output_dir, device_ids):
        import jax

        jax.devices()
        if device_ids:
            ids = (ctypes.c_int64 * len(device_ids))(*device_ids)
            rc = lib.axon_start_nrt_profile(ids, len(device_ids))
        else:
            rc = lib.axon_start_nrt_profile(None, 0)
        if rc != 0:
            raise RuntimeError(f"axon_start_nrt_profile rc={rc}")
        try:
            yield
        finally:
            n = lib.axon_stop_nrt_profile(str(output_dir).encode())
            print(f"ntff profile: {n} file(s) written to {output_dir}")

    mod.set_axon_ntff_profile_hook(_hook)


_NC_CACHE = None


def _get_module():
    global _NC_CACHE
    if _NC_CACHE is None:
        _NC_CACHE = _build_module()
    return _NC_CACHE


def _run(x, weights, coeff, trace=False):
    nc = _get_module()
    r, bias = _prep_operands(weights, coeff)
    xs = _prep_x(np.asarray(x, dtype=np.float32))
    in_maps = [{"xt": xs[core], "r": r} for core in range(NCORES)]
    try:
        res = run_bass_kernel_spmd(
            nc, in_maps, core_ids=list(range(NCORES)), trace=trace
        )
    except Exception:
        res = run_bass_kernel_spmd(
            nc, in_maps, core_ids=list(range(NCORES)), trace=trace
        )
    out = np.concatenate(
        [
            res.results[core]["out"]
            .astype(np.float32)
            .reshape(P, BT, O)
            .transpose(1, 0, 2)
            .reshape(BC, O)
            for core in range(NCORES)
        ],
        axis=0,
    )
    out += bias[None, :]
    return out, res


def kernel(x, weights, coeff):
    out, _ = _run(x, weights, coeff, trace=False)
    return out


def kernel_traced(x, weights, coeff):
    _install_ntff_hook()
    out, res = _run(x, weights, coeff, trace=True)
    return out, res
